# revision 9
# baseline (speedup 1.0000x reference)
"""Trainium2 Bass kernel for nn_Attention_45930380263558.

Attention module (EfficientViT-style attention with a gathered relative
position bias) over x:[16, 1024, 512]:
    qkv = x @ qkv_w + qkv_b                  # [B, N, 2048]
    split per head h: q,k (64), v (128)
    attn = softmax(q k^T * 64^-0.5 + bias_h[gather])
    out  = (attn @ v) per head, concat -> @ proj_w + proj_b

Sharding: data-parallel over batch, 2 batches per core on 8 NeuronCores.
No collectives. Each core computes its 2 batches fully.

Per-core dataflow (everything fp32, matmuls via float32r bitcast — full
PE rate for free dim >= 256):
  - QKV^T form for q,k: QK^T[ch, tok] = W_qk^T chunks (stationary) x
    x^T (moving). Channel layout: all q channels first (h-major), then
    all k channels, so Q_h^T and K_h^T always share a base partition.
  - V in natural [tok, vch] form: x^T chunk stationary, W_v moving.
  - Per (batch, head): S^T[k, q] = K_h^T.T @ Q_h^T (K=64 contraction),
    E = exp(SCALE * S^T) on ScalarE (PSUM->SBUF), P^T = E * expB_h
    (host-precomputed exp of the gathered bias, fp16) on VectorE,
    O'^T[v, q] += V_chunk.T @ P^T chunks (PSUM accum), rowsum[q] +=
    ones.T @ P^T chunks, then 1/rowsum broadcast to 128 partitions via a
    K=1 ones matmul, O_norm^T = O'^T * bcast, proj partial = O_norm^T
    chunks (stationary) x proj_w rows of head h, accumulated in SBUF on
    VectorE. Softmax max-subtraction is skipped (logits bounded ~|7|,
    mathematically identical after normalization).
"""

import os
import sys

for _p in ("/opt/trn_rl_repo",):
    if _p not in sys.path and os.path.isdir(_p):
        sys.path.insert(0, _p)

from contextlib import ExitStack

import numpy as np

import concourse.bass as bass
import concourse.tile as tile
from concourse import bacc, mybir
from concourse.bass_utils import run_bass_kernel_spmd

F32 = mybir.dt.float32
F32R = mybir.dt.float32r
F16 = mybir.dt.float16

# exp-bias table dtype: "f32" (exact) or "f16" (half DMA traffic, ~2e-4 rel err)
EB_DTYPE = os.environ.get("EB_DTYPE", "f32")

N_CORES = 8
B = 16
B_LOC = B // N_CORES  # 2
N = 1024  # tokens
D = 512  # model dim
H = 8  # heads
DK = 64  # key dim
DV = 128  # value dim per head
SCALE = DK ** -0.5
NT = N // 128  # 8 token tiles
DC = D // 128  # 4 dim chunks
QH = 2  # q halves of 512

# module-level stash so test.py can read timing info
LAST_RESULT = None


def _ensure_axon_hooks_module():
    """bass_utils' trace path imports antenv.axon_hooks, which some agent
    images lack. Provide a minimal get/set pair so trace degrades
    gracefully (hook=None -> tracing skipped) instead of crashing."""
    try:
        import antenv.axon_hooks  # noqa: F401
        return
    except ImportError:
        pass
    import types

    import antenv

    m = types.ModuleType("antenv.axon_hooks")
    m._hook = None

    def set_axon_ntff_profile_hook(h):
        m._hook = h

    def get_axon_ntff_profile_hook():
        return m._hook

    m.set_axon_ntff_profile_hook = set_axon_ntff_profile_hook
    m.get_axon_ntff_profile_hook = get_axon_ntff_profile_hook
    sys.modules["antenv.axon_hooks"] = m
    antenv.axon_hooks = m


_ensure_axon_hooks_module()


def build_program(use_qkv_bias: bool, use_proj_bias: bool, trace: bool = False):
    nc = bacc.Bacc("TRN2", target_bir_lowering=False, debug=False,
                   num_devices=N_CORES)

    xT_d = nc.dram_tensor("xT", [B_LOC, DC, 128, N], F32R, kind="ExternalInput").ap()
    w_qk_d = nc.dram_tensor("w_qk", [DC, 128, N], F32R, kind="ExternalInput").ap()
    w_v_d = nc.dram_tensor("w_v", [DC, 128, N], F32R, kind="ExternalInput").ap()
    eb_dt = F16 if EB_DTYPE == "f16" else F32
    eb_d = nc.dram_tensor("eb", [H, NT, 128, N], eb_dt, kind="ExternalInput").ap()
    w_proj_d = nc.dram_tensor("w_proj", [H, 128, D], F32R, kind="ExternalInput").ap()
    ones_d = nc.dram_tensor("ones", [128, N], F32R, kind="ExternalInput").ap()
    out_d = nc.dram_tensor("out", [B_LOC, N, D], F32, kind="ExternalOutput").ap()
    if use_qkv_bias:
        qk_bias_d = nc.dram_tensor("qk_bias", [1, N], F32R, kind="ExternalInput").ap()
        v_bias_d = nc.dram_tensor("v_bias", [1, N], F32R, kind="ExternalInput").ap()
    if use_proj_bias:
        proj_bias_d = nc.dram_tensor("proj_bias", [1, D], F32R, kind="ExternalInput").ap()

    with tile.TileContext(nc) as tc, ExitStack() as ctx:
        consts = ctx.enter_context(tc.tile_pool(name="consts", bufs=1))
        xp = ctx.enter_context(tc.tile_pool(name="xp", bufs=1))
        qkp = ctx.enter_context(tc.tile_pool(name="qkp", bufs=1))
        vp = ctx.enter_context(tc.tile_pool(name="vp", bufs=1))
        ebp = ctx.enter_context(tc.tile_pool(name="ebp", bufs=2))
        ep = ctx.enter_context(tc.tile_pool(name="ep", bufs=2))
        ptp = ctx.enter_context(tc.tile_pool(name="ptp", bufs=2))
        bcp = ctx.enter_context(tc.tile_pool(name="bcp", bufs=1))
        onp = ctx.enter_context(tc.tile_pool(name="onp", bufs=1))
        accp = ctx.enter_context(tc.tile_pool(name="accp", bufs=1))
        smallp = ctx.enter_context(tc.tile_pool(name="smallp", bufs=1))

        ps_s = ctx.enter_context(tc.tile_pool(name="ps_s", bufs=2, space="PSUM"))
        ps_o = ctx.enter_context(tc.tile_pool(name="ps_o", bufs=1, space="PSUM"))
        ps_rs = ctx.enter_context(tc.tile_pool(name="ps_rs", bufs=1, space="PSUM"))

        # constants
        w_qk_t = consts.tile([128, DC, N], F32R)
        nc.sync.dma_start(out=w_qk_t, in_=w_qk_d.transpose([1, 0, 2]))
        w_v_t = consts.tile([128, DC, N], F32R)
        nc.sync.dma_start(out=w_v_t, in_=w_v_d.transpose([1, 0, 2]))
        w_proj_t = consts.tile([128, H, D], F32R)
        nc.sync.dma_start(out=w_proj_t, in_=w_proj_d.transpose([1, 0, 2]))
        ones_t = consts.tile([128, N], F32R)
        nc.sync.dma_start(out=ones_t, in_=ones_d)
        ones_col = ones_t[:, 0:1]
        ones_row = ones_t[0:1, 0:128]
        if use_qkv_bias:
            qk_bias_t = consts.tile([1, N], F32R)
            nc.sync.dma_start(out=qk_bias_t, in_=qk_bias_d)
            v_bias_t = consts.tile([1, N], F32R)
            nc.sync.dma_start(out=v_bias_t, in_=v_bias_d)
            ones_n = ones_t[0:1, :]
        if use_proj_bias:
            proj_bias_t = consts.tile([1, D], F32R)
            nc.sync.dma_start(out=proj_bias_t, in_=proj_bias_d)

        for b in range(B_LOC):
            # ---- load x^T chunks ----
            x_t = xp.tile([128, DC, N], F32R)
            nc.sync.dma_start(out=x_t, in_=xT_d[b].transpose([1, 0, 2]))

            # ---- Form1: QK^T[ch, tok] ----
            # M-tiles 0..3 -> q channels (heads 2mt, 2mt+1), 4..7 -> k channels
            qk_sb = qkp.tile([128, NT, N], F32R)
            for mt in range(NT):
                w_col = w_qk_t[:, :, mt * 128:(mt + 1) * 128]
                for nt in range(QH):
                    pp = ps_s.tile([128, 512], F32, tag="s")
                    for kc in range(DC):
                        nc.tensor.matmul(
                            pp,
                            lhsT=(w_col[:, kc, :]),
                            rhs=(x_t[:, kc, nt * 512:(nt + 1) * 512]),
                            start=(kc == 0),
                            stop=(kc == DC - 1 and not use_qkv_bias),
                        )
                    if use_qkv_bias:
                        nc.tensor.matmul(
                            pp,
                            lhsT=(qk_bias_t[:, mt * 128:(mt + 1) * 128]),
                            rhs=(ones_n[:, nt * 512:(nt + 1) * 512]),
                            start=False, stop=True,
                        )
                    nc.scalar.copy(qk_sb[:, mt, nt * 512:(nt + 1) * 512], pp)

            # ---- Form2: V[tok, vch] ----
            v_sb = vp.tile([128, NT, N], F32R)
            for tt in range(NT):
                for nt in range(QH):
                    pp = ps_s.tile([128, 512], F32, tag="s")
                    for kc in range(DC):
                        nc.tensor.matmul(
                            pp,
                            lhsT=(x_t[:, kc, tt * 128:(tt + 1) * 128]),
                            rhs=(w_v_t[:, kc, nt * 512:(nt + 1) * 512]),
                            start=(kc == 0),
                            stop=(kc == DC - 1 and not use_qkv_bias),
                        )
                    if use_qkv_bias:
                        nc.tensor.matmul(
                            pp,
                            lhsT=(ones_n[:, tt * 128:(tt + 1) * 128]),
                            rhs=(v_bias_t[:, nt * 512:(nt + 1) * 512]),
                            start=False, stop=True,
                        )
                    nc.scalar.copy(v_sb[:, tt, nt * 512:(nt + 1) * 512], pp)

            # ---- attention per head ----
            acc_sb = accp.tile([128, NT, D], F32)  # [q-tile partitions, qt, d]
            for h in range(H):
                par = (h % 2) * 64
                qT = qk_sb[par:par + 64, h // 2, :]       # [64, 1024]
                kT = qk_sb[par:par + 64, 4 + h // 2, :]   # [64, 1024]

                o_ps = ps_o.tile([128, N], F32)           # [v, q] accum
                rs_ps = ps_rs.tile([1, N], F32)           # rowsum accum

                for kc in range(NT):
                    # eb chunk (two kc per DMA)
                    if kc % 2 == 0:
                        eb_t = ebp.tile([128, 2, N], eb_dt)
                        nc.sync.dma_start(
                            out=eb_t,
                            in_=eb_d[h].transpose([1, 0, 2])[:, kc:kc + 2, :],
                        )
                    st = ps_s.tile([128, N], F32, tag="s")
                    for qh in range(QH):
                        nc.tensor.matmul(
                            st[:, qh * 512:(qh + 1) * 512],
                            lhsT=(kT[:, kc * 128:(kc + 1) * 128]),
                            rhs=(qT[:, qh * 512:(qh + 1) * 512]),
                            start=True, stop=True,
                        )
                    e_t = ep.tile([128, N], F32)
                    nc.scalar.activation(e_t, st,
                                         mybir.ActivationFunctionType.Exp,
                                         scale=SCALE)
                    pt_t = ptp.tile([128, N], F32R)
                    nc.vector.tensor_tensor(pt_t, e_t, eb_t[:, kc % 2, :],
                                            op=mybir.AluOpType.mult)
                    v_head = v_sb[:, kc, h * 128:(h + 1) * 128]  # [128 tok, 128 v]
                    for qh in range(QH):
                        nc.tensor.matmul(
                            o_ps[:, qh * 512:(qh + 1) * 512],
                            lhsT=(v_head),
                            rhs=(pt_t[:, qh * 512:(qh + 1) * 512]),
                            start=(kc == 0), stop=(kc == NT - 1),
                        )
                        nc.tensor.matmul(
                            rs_ps[:, qh * 512:(qh + 1) * 512],
                            lhsT=(ones_col),
                            rhs=(pt_t[:, qh * 512:(qh + 1) * 512]),
                            start=(kc == 0), stop=(kc == NT - 1),
                        )

                # 1/rowsum, broadcast to 128 partitions
                inv_t = smallp.tile([1, N], F32R)
                with nc.allow_low_precision(reason="1/rowsum feeds fp32r bcast matmul"):
                    nc.vector.reciprocal(inv_t, rs_ps)
                bc_ps = ps_s.tile([128, N], F32, tag="s")
                for qh in range(QH):
                    nc.tensor.matmul(
                        bc_ps[:, qh * 512:(qh + 1) * 512],
                        lhsT=(ones_row),
                        rhs=(inv_t[:, qh * 512:(qh + 1) * 512]),
                        start=True, stop=True,
                    )
                bc_t = bcp.tile([128, N], F32)
                nc.scalar.copy(bc_t, bc_ps)
                on_t = onp.tile([128, N], F32R)
                nc.vector.tensor_tensor(on_t, o_ps, bc_t,
                                        op=mybir.AluOpType.mult)

                # proj partial for this head, accumulate in SBUF
                for qt in range(NT):
                    pp = ps_s.tile([128, D], F32, tag="s")
                    last = (h == H - 1)
                    nc.tensor.matmul(
                        pp,
                        lhsT=(on_t[:, qt * 128:(qt + 1) * 128]),
                        rhs=(w_proj_t[:, h, :]),
                        start=True, stop=not (last and use_proj_bias),
                    )
                    if last and use_proj_bias:
                        nc.tensor.matmul(
                            pp,
                            lhsT=(ones_row),
                            rhs=(proj_bias_t),
                            start=False, stop=True,
                        )
                    if h == 0:
                        nc.vector.tensor_copy(acc_sb[:, qt, :], pp)
                    else:
                        nc.vector.tensor_tensor(acc_sb[:, qt, :], pp,
                                                acc_sb[:, qt, :],
                                                op=mybir.AluOpType.add)

            # ---- store output ----
            for qt in range(NT):
                nc.sync.dma_start(
                    out=out_d[b, qt * 128:(qt + 1) * 128, :],
                    in_=acc_sb[:, qt, :],
                )

    nc.compile()
    return nc


def _prep_core_inputs(x, qkv_w, qkv_b, proj_w, proj_b, attn_biases, bias_idxs):
    """Host-side layout preparation. Returns (shared, per_core_xT, flags)."""
    x = np.ascontiguousarray(np.asarray(x, np.float32))
    qkv_w = np.asarray(qkv_w, np.float32)
    qkv_b = np.asarray(qkv_b, np.float32)
    proj_w = np.asarray(proj_w, np.float32)
    proj_b = np.asarray(proj_b, np.float32)
    attn_biases = np.asarray(attn_biases, np.float32)
    bias_idxs = np.asarray(bias_idxs)

    # qkv_w columns: per head 256 = [q 64 | k 64 | v 128]
    Wh = qkv_w.reshape(D, H, 256)
    w_q = Wh[:, :, :DK].transpose(0, 1, 2).reshape(D, H * DK)       # q chans h-major
    w_k = Wh[:, :, DK:2 * DK].reshape(D, H * DK)
    w_qk = np.concatenate([w_q, w_k], axis=1)                        # [512, 1024]
    w_v = Wh[:, :, 2 * DK:].reshape(D, H * DV)                       # [512, 1024]

    bh = qkv_b.reshape(H, 256)
    qk_bias = np.concatenate([bh[:, :DK].reshape(-1), bh[:, DK:2 * DK].reshape(-1)])
    v_bias = bh[:, 2 * DK:].reshape(-1)

    # gathered bias, exp'd, transposed to [H, k, q], fp16, tiled [H, NT, 128, N]
    EB = np.exp(attn_biases[:, bias_idxs])                           # [H, q, k]
    EBt = np.ascontiguousarray(EB.transpose(0, 2, 1))                # [H, k, q]
    eb = EBt.reshape(H, NT, 128, N).astype(
        np.float16 if EB_DTYPE == "f16" else np.float32)

    shared = {
        "ones": np.ones((128, N), np.float32),
        "w_qk": np.ascontiguousarray(w_qk.reshape(DC, 128, H * DK * 2)),
        "w_v": np.ascontiguousarray(w_v.reshape(DC, 128, H * DV)),
        "eb": eb,
        "w_proj": np.ascontiguousarray(proj_w.reshape(H, 128, D)),
    }
    use_qkv_bias = bool(np.any(qkv_b))
    use_proj_bias = bool(np.any(proj_b))
    if use_qkv_bias:
        shared["qk_bias"] = qk_bias.reshape(1, N)
        shared["v_bias"] = v_bias.reshape(1, N)
    if use_proj_bias:
        shared["proj_bias"] = proj_b.reshape(1, D)

    # x^T per core: [B_LOC, DC, 128, N]
    xT = np.ascontiguousarray(x.transpose(0, 2, 1)).reshape(B, DC, 128, N)
    per_core = [xT[c * B_LOC:(c + 1) * B_LOC] for c in range(N_CORES)]
    return shared, per_core, use_qkv_bias, use_proj_bias


def kernel(x, qkv_w, qkv_b, proj_w, proj_b, attn_biases, bias_idxs):
    global LAST_RESULT
    shared, per_core, use_qkv_bias, use_proj_bias = _prep_core_inputs(
        x, qkv_w, qkv_b, proj_w, proj_b, attn_biases, bias_idxs)

    nc = build_program(use_qkv_bias, use_proj_bias)

    in_maps = [dict(shared, xT=per_core[c]) for c in range(N_CORES)]
    trace = bool(os.environ.get("BASS_TRACE"))
    res = run_bass_kernel_spmd(nc, in_maps, core_ids=list(range(N_CORES)),
                               trace=trace)
    LAST_RESULT = res
    out = np.concatenate([res.results[c]["out"] for c in range(N_CORES)], axis=0)
    return np.ascontiguousarray(out.astype(np.float32))


if __name__ == "__main__":
    # smoke test in CoreSim for core 0 (set SIM=1)
    pass
